# revision 1
# baseline (speedup 1.0000x reference)
"""Cross-attention kernel for Trainium2 (8 NeuronCores, SPMD data-parallel).

Problem: B=4, C=128, 64x64 spatial (N=4096 tokens), 4 heads of dim 32.
  q = Wq @ query; k = Wk @ key; v = Wv @ key   (1x1 convs == channel matmuls)
  out = softmax(q^T k / sqrt(32)) @ v          (per batch*head)

Sharding: 16 (batch, head) jobs -> 2 per core. Core i handles batch i//2,
heads {2*(i%2), 2*(i%2)+1} i.e. output channels [64*(i%2), 64*(i%2)+64).

On-chip layout ("scoresT"): scores are computed transposed, [nk, nq], so that
the PV matmul needs no transposes and the context comes out directly in
channel-major [d, nq] output layout. The softmax denominator is computed by
appending a ones-column to v^T (rides the PV accumulation for free); the
final division happens on the host (softmax is scale-invariant so exp-max
subtraction is unnecessary: scores ~ N(0,1)).

dtypes: fp32 matmuls run at 1/4 rate on TRN2 (two half-speed passes), so
everything the PE touches is bf16 (inputs cast on host; engine copies round
on write; PSUM accumulation stays fp32).

exp() is split between ScalarE (exact spline exp -> bf16) and VectorE
(single-op Schraudolph exp2: int16 <- y*128 + B, bits reinterpreted as bf16),
because the PSUM->SBUF drain of the 33.5M score elements per core is the
throughput floor and ACT alone would take ~220us.
"""

import functools
import math

import numpy as np

NCORES = 8
B, C, HS, WS = 4, 128, 64, 64
N = HS * WS  # 4096 tokens
NUM_HEADS = 4
DH = 32  # head dim
HPC = 2  # heads per core

NQB = 512  # nq block (PSUM bank = 512 f32)
NKC = 128  # nk chunk (matmul K tile)
N_BLOCKS = N // NQB  # 8
N_CHUNKS = N // NKC  # 32
VTW = 66  # v^T tile width: 32 v cols + 1 ones col + pad to >64 so that
#            round_up(M)=128 keeps every matmul in the same 128x128 PE mode

# Schraudolph exp2 in bf16: i16 = cvt(y*128 + (16256 - C)); bits = bf16 ~ 2^y
# scores are pre-scaled by log2(e)/sqrt(DH) on the Wq side so y is in log2
# domain. ACT chunks use Exp with scale=ln(2) to undo the log2 scaling.
EXP2_A = 128.0
EXP2_B = 16256.0 - 5.25
DVE_EXP = True


def _f32(x):
    return np.ascontiguousarray(np.asarray(x, dtype=np.float32))


def _bf16(x):
    import ml_dtypes

    return np.ascontiguousarray(np.asarray(x, dtype=np.float32).astype(ml_dtypes.bfloat16))


@functools.lru_cache(maxsize=1)
def _build_program():
    from contextlib import ExitStack

    import concourse.tile as tile
    from concourse import bacc, mybir
    from concourse.bass import ts

    f32 = mybir.dt.float32
    bf16 = mybir.dt.bfloat16
    i16 = mybir.dt.int16
    AF = mybir.ActivationFunctionType
    ALU = mybir.AluOpType

    nc = bacc.Bacc(
        "TRN2",
        target_bir_lowering=False,
        debug=False,
        enable_asserts=False,
        num_devices=NCORES,
    )

    qin = nc.dram_tensor("qin", [128, N], bf16, kind="ExternalInput").ap()
    kin = nc.dram_tensor("kin", [128, N], bf16, kind="ExternalInput").ap()
    # weight layouts (host-prepared):
    #  wq_t[:, 128*h + d] = Wq[ch(h,d), :] * log2e/sqrt(DH), cols 32..127 of
    #  each 128-block are ZERO -> projection output rows 32..127 are zeros,
    #  which lets every matmul use K=128 (uniform PE mode, zero-padded).
    wq_t = nc.dram_tensor("wq_t", [128, 256], bf16, kind="ExternalInput").ap()
    wk_t = nc.dram_tensor("wk_t", [128, 256], bf16, kind="ExternalInput").ap()
    wv_t = nc.dram_tensor("wv_t", [128, 64], bf16, kind="ExternalInput").ap()

    out_ctx = nc.dram_tensor("out_ctx", [64, N], f32, kind="ExternalOutput").ap()
    out_den = nc.dram_tensor("out_den", [2, N], f32, kind="ExternalOutput").ap()

    with tile.TileContext(nc) as tc, ExitStack() as ctx:
        persist = ctx.enter_context(tc.tile_pool(name="persist", bufs=1))

        # ---- load inputs ----
        wq_sb = persist.tile([128, 256], bf16)
        wk_sb = persist.tile([128, 256], bf16)
        wv_sb = persist.tile([128, 64], bf16)
        nc.sync.dma_start(out=wq_sb, in_=wq_t)
        nc.sync.dma_start(out=wk_sb, in_=wk_t)
        nc.sync.dma_start(out=wv_sb, in_=wv_t)

        # ---- projections: qz/kz[h] = [128, N] bf16, rows 0-31 = per-head
        # q/k^T (d on partitions), rows 32-127 exact zeros ----
        qzb = persist.tile([128, N], bf16, name="qzb")
        kz = [persist.tile([128, N], bf16, name=f"kz{h}") for h in range(HPC)]
        # v^T per head: chunk c occupies cols [c*VTW, c*VTW+32) (nk on
        # partitions), col c*VTW+32 is the ones column for the denominator.
        vt = [
            persist.tile([128, VTW * N_CHUNKS], bf16, name=f"vt{h}")
            for h in range(HPC)
        ]
        for h in range(HPC):
            nc.vector.memset(vt[h], 1.0)

        # One shared PSUM pool for projection outputs and attention score
        # tiles (same 2-bank slot size): a separate proj pool would force the
        # whole attention phase to wait for its release barrier.
        sc_pool = ctx.enter_context(tc.tile_pool(name="sc", bufs=3, space="PSUM"))
        ctx_pool = ctx.enter_context(tc.tile_pool(name="ctxp", bufs=2, space="PSUM"))
        ex_pool = ctx.enter_context(tc.tile_pool(name="ex", bufs=8))
        out_pool = ctx.enter_context(tc.tile_pool(name="outp", bufs=4))

        with tc.tile_pool(name="inp", bufs=1) as inp_pool:
            qin_sb = inp_pool.tile([128, N], bf16)
            kin_sb = inp_pool.tile([128, N], bf16)
            # alternate chunks across the HWDGE (sync) and SWDGE (gpsimd)
            # queues so q and k stream in concurrently
            for t in range(4):
                eng_q = nc.sync if t % 2 == 0 else nc.gpsimd
                eng_k = nc.gpsimd if t % 2 == 0 else nc.sync
                eng_q.dma_start(
                    out=qin_sb[:, ts(t, N // 4)], in_=qin[:, ts(t, N // 4)]
                )
                eng_k.dma_start(
                    out=kin_sb[:, ts(t, N // 4)], in_=kin[:, ts(t, N // 4)]
                )

            pp = sc_pool
            for t2 in range(N // (2 * NQB)):
                qp = pp.tile([128, 2 * NQB], f32, name="qp", tag="sc")
                for j in range(2):
                    nc.tensor.matmul(
                        out=qp[:, ts(j, NQB)],
                        lhsT=wq_sb[:, 0:128],
                        rhs=qin_sb[:, ts(2 * t2 + j, NQB)],
                        start=True,
                        stop=True,
                    )
                eng = nc.vector.tensor_copy if t2 % 2 == 0 else nc.scalar.copy
                eng(qzb[:, ts(t2, 2 * NQB)], qp)
            for h in range(HPC):
                for t2 in range(N // (2 * NQB)):
                    kp = pp.tile([128, 2 * NQB], f32, name="kp", tag="sc")
                    for j in range(2):
                        nc.tensor.matmul(
                            out=kp[:, ts(j, NQB)],
                            lhsT=wk_sb[:, ts(h, 128)],
                            rhs=kin_sb[:, ts(2 * t2 + j, NQB)],
                            start=True,
                            stop=True,
                        )
                    eng = nc.vector.tensor_copy if t2 % 2 == 0 else nc.scalar.copy
                    eng(kz[h][:, ts(t2, 2 * NQB)], kp)

            # v^T: out[nk, c_out] = sum_c key[c, nk] * Wv[c_out, c]
            # 16 chunks of [128, 64] fill one 2-bank psum tile; copy per head.
            for g in range(N_CHUNKS // 16):
                vp = pp.tile([128, 2 * NQB], f32, name="vp", tag="sc")
                for j in range(16):
                    c = g * 16 + j
                    nc.tensor.matmul(
                        out=vp[:, ts(j, 64)],
                        lhsT=kin_sb[:, ts(c, NKC)],
                        rhs=wv_sb,
                        start=True,
                        stop=True,
                    )
                vp3 = vp.rearrange("p (j w) -> p j w", j=16)
                for h in range(HPC):
                    dst = vt[h][:, g * 16 * VTW : (g + 1) * 16 * VTW]
                    dst3 = dst.rearrange("p (j w) -> p j w", j=16)
                    eng = nc.vector.tensor_copy if h % 2 == 0 else nc.scalar.copy
                    eng(dst3[:, :, 0:32], vp3[:, :, ts(h, 32)])

        # ---- attention ----
        # sc tiles span 2 PSUM banks (2 nk chunks); one exp instruction
        # drains both chunks to amortize the per-op overhead.
        ln2 = math.log(2.0)
        for h in range(HPC):
            for b in range(N_BLOCKS):
                ctx_ps = ctx_pool.tile([VTW, NQB], f32, name="ctx_ps")
                for c2 in range(N_CHUNKS // 2):
                    sc = sc_pool.tile([128, 2 * NQB], f32, name="sc")
                    for j in range(2):
                        # scoresT[nk_chunk, nq_block] = k^T q (K=d, 0-padded)
                        nc.tensor.matmul(
                            out=sc[:, ts(j, NQB)],
                            lhsT=kz[h][:, ts(2 * c2 + j, NKC)],
                            rhs=qzb[:, ts(b, NQB)],
                            start=True,
                            stop=True,
                        )
                    ex = ex_pool.tile([128, 2 * NQB], bf16, name="ex")
                    if DVE_EXP and c2 % 2 == 1:
                        # fast exp2 on DVE: i16 <- sc*128 + bias; bits = bf16
                        nc.vector.tensor_scalar(
                            ex.bitcast(i16), sc, EXP2_A, EXP2_B,
                            op0=ALU.mult, op1=ALU.add,
                        )
                    else:
                        # exact exp on ACT: exp(ln2 * y) = 2^y
                        nc.scalar.activation(ex, sc, AF.Exp, scale=ln2)
                    for j in range(2):
                        # ctxT[d, nq] += v^T[d, nk] @ probsT[nk, nq]; row 32
                        # accumulates the softmax denominator (ones column).
                        c = 2 * c2 + j
                        nc.tensor.matmul(
                            out=ctx_ps,
                            lhsT=vt[h][:, c * VTW : (c + 1) * VTW],
                            rhs=ex[:, ts(j, NQB)],
                            start=(c == 0),
                            stop=(c == N_CHUNKS - 1),
                        )
                ob = out_pool.tile([33, NQB], f32, name="ob")
                nc.scalar.copy(ob, ctx_ps[0:33, :])
                nc.sync.dma_start(out=out_ctx[ts(h, 32), ts(b, NQB)], in_=ob[0:32, :])
                nc.sync.dma_start(
                    out=out_den[h : h + 1, ts(b, NQB)], in_=ob[32:33, :]
                )

    nc.compile()
    return nc


def _shard_inputs(query, key, Wq, Wk, Wv):
    query = _f32(query).reshape(B, C, N)
    key = _f32(key).reshape(B, C, N)
    Wq, Wk, Wv = _f32(Wq), _f32(Wk), _f32(Wv)

    scale = math.log2(math.e) / math.sqrt(DH)
    in_maps = []
    for core in range(NCORES):
        b, half = core // 2, core % 2
        wq_t = np.zeros((128, 256), np.float32)
        wk_t = np.zeros((128, 256), np.float32)
        wv_t = np.zeros((128, 64), np.float32)
        for hl in range(HPC):
            ch0 = 64 * half + 32 * hl
            # merged qzb: h0 q at rows 0-31, h1 q at rows 32-63; per-head
            # kz carries the head separation, so head h's k-weights sit at
            # column band 32h inside its 128-block (contraction-row align)
            wq_t[:, 32 * hl : 32 * hl + 32] = Wq[ch0 : ch0 + 32, :].T * scale
            wk_t[:, 128 * hl + 32 * hl : 128 * hl + 32 * hl + 32] = (
                Wk[ch0 : ch0 + 32, :].T
            )
            wv_t[:, 32 * hl : 32 * hl + 32] = Wv[ch0 : ch0 + 32, :].T
        in_maps.append(
            {
                "qin": _bf16(query[b]),
                "kin": _bf16(key[b]),
                "wq_t": _bf16(wq_t),
                "wk_t": _bf16(wk_t),
                "wv_t": _bf16(wv_t),
            }
        )
    return in_maps


def _run(in_maps, trace=False):
    from concourse import bass_utils

    nc = _build_program()
    return bass_utils.run_bass_kernel_spmd(
        nc, in_maps, core_ids=list(range(NCORES)), trace=trace
    )


def _assemble(results):
    out = np.empty((B, C, N), np.float32)
    for core in range(NCORES):
        b, half = core // 2, core % 2
        r = results[core]
        ctx = r["out_ctx"]  # [64, N]
        den = r["out_den"]  # [2, N]
        for hl in range(HPC):
            out[b, 64 * half + 32 * hl : 64 * half + 32 * hl + 32, :] = (
                ctx[32 * hl : 32 * hl + 32, :] / den[hl][None, :]
            )
    return out.reshape(B, C, HS, WS)


def kernel(query, key, Wq, Wk, Wv):
    in_maps = _shard_inputs(query, key, Wq, Wk, Wv)
    res = _run(in_maps)
    return _assemble(res.results)



# revision 7
# speedup vs baseline: 1.2821x; 1.2821x over previous
"""Cross-attention kernel for Trainium2 (8 NeuronCores, SPMD data-parallel).

Problem: B=4, C=128, 64x64 spatial (N=4096 tokens), 4 heads of dim 32.
  q = Wq @ query; k = Wk @ key; v = Wv @ key   (1x1 convs == channel matmuls)
  out = softmax(q^T k / sqrt(32)) @ v          (per batch*head)

Sharding: 16 (batch, head) jobs -> 2 per core. Core i handles batch i//2,
heads {2*(i%2), 2*(i%2)+1} i.e. output channels [64*(i%2), 64*(i%2)+64).

Structure (per core):
  - The k-projection is folded into the q side on the host:
    scoresT = kin^T (M_h qin) with M_h = log2(e)/sqrt(32) * Wk_h^T Wq_h,
    so raw kin serves as the QK lhsT (no k-projection on device) and only
    t_h = M_h qin (the "q" side, full 128 rows) is projected and cast.
  - QK: scoresT[nk_chunk=128, nq_block=512] = kin_chunk(lhsT) @ t_block,
    K=128 contraction, bf16, scores arrive in the log2 domain.
  - exp: PSUM->SBUF drain split between DVE (one-op Schraudolph exp2:
    int16 <- y*128 + bias, bits reinterpreted as bf16) and ACT (exact
    table exp with scale=ln2), weighted by modeled engine rates.
  - PV flipped: ctx[nq=128, 33] += probsT_chunk(lhsT, stationary) @ v_aug
    (moving, 32 v columns + 1 ones column for the softmax denominator), so
    each chunk's matmul streams only 33 columns instead of 512. v itself is
    projected flipped (v^T[nk, d] = kin_chunk^T(lhsT) @ Wv_h, 32 cols).
  - host: normalize by the denominator column and transpose to [C, N].
"""

import functools
import math

import numpy as np

NCORES = 8
B, C, HS, WS = 4, 128, 64, 64
N = HS * WS  # 4096 tokens
NUM_HEADS = 4
DH = 32  # head dim
HPC = 2  # heads per core

NQB = 512  # nq per QK matmul (one PSUM bank of f32)
NKC = 128  # nk chunk (PV contraction tile)
N_BLOCKS = N // NQB  # 8
N_CHUNKS = N // NKC  # 32
VTW = 33  # v^T tile width: 32 v cols + 1 ones col (denominator)

# Schraudolph exp2 in bf16: i16 = cvt(y*128 + (16256 - C)); bits = bf16 ~ 2^y
EXP2_A = 128.0
EXP2_B = 16256.0 - 5.25

# exp work split across ACT/DVE proportional to modeled per-tile rates
# (GPSIMD cannot access PSUM per the BIR verifier)
EXP_W = {"A": 1.0 / 1038.0, "D": 1.0 / 1192.0}


def _exp_schedule(n):
    """Weighted round-robin assignment of n exp tiles to engines."""
    acc = {k: 0.0 for k in EXP_W}
    tot = sum(EXP_W.values())
    out = []
    for _ in range(n):
        for k in EXP_W:
            acc[k] += EXP_W[k] / tot
        pick = max(acc, key=lambda k: acc[k])
        acc[pick] -= 1.0
        out.append(pick)
    return out


def _f32(x):
    return np.ascontiguousarray(np.asarray(x, dtype=np.float32))


def _bf16(x):
    import ml_dtypes

    return np.ascontiguousarray(
        np.asarray(x, dtype=np.float32).astype(ml_dtypes.bfloat16)
    )


@functools.lru_cache(maxsize=1)
def _build_program():
    from contextlib import ExitStack

    import concourse.tile as tile
    from concourse import bacc, mybir
    from concourse.bass import ts

    f32 = mybir.dt.float32
    bf16 = mybir.dt.bfloat16
    i16 = mybir.dt.int16
    AF = mybir.ActivationFunctionType
    ALU = mybir.AluOpType

    nc = bacc.Bacc(
        "TRN2",
        target_bir_lowering=False,
        debug=False,
        enable_asserts=False,
        num_devices=NCORES,
    )

    qin = nc.dram_tensor("qin", [128, N], bf16, kind="ExternalInput").ap()
    kin = nc.dram_tensor("kin", [128, N], bf16, kind="ExternalInput").ap()
    # win: [m_h0 (128) | m_h1 (128) | wv_h0 (32) | wv_h1 (32)]
    # m_h = log2(e)/sqrt(DH) * Wq_h^T @ Wk_h  (used as lhsT for the t-proj)
    win = nc.dram_tensor("win", [128, 320], bf16, kind="ExternalInput").ap()

    # per (h, nq-block): ctx rows [nq=128 x 4 j-tiles], cols 32 ctx + 1 den
    out_ctx = nc.dram_tensor(
        "out_ctx", [HPC * N_BLOCKS, 128, 4 * VTW], f32, kind="ExternalOutput"
    ).ap()

    ln2 = math.log(2.0)

    with tile.TileContext(nc) as tc, ExitStack() as ctx:
        persist = ctx.enter_context(tc.tile_pool(name="persist", bufs=1))

        win_sb = persist.tile([128, 320], bf16)
        nc.sync.dma_start(out=win_sb, in_=win)

        qin_sb = persist.tile([128, N], bf16)
        kin_sb = persist.tile([128, N], bf16)
        for t in range(4):
            nc.sync.dma_start(
                out=qin_sb[:, ts(t, N // 4)], in_=qin[:, ts(t, N // 4)]
            )
            nc.sync.dma_start(
                out=kin_sb[:, ts(t, N // 4)], in_=kin[:, ts(t, N // 4)]
            )

        # persistent projections: t_h = M_h qin (q side), v^T per head
        tz = [persist.tile([128, N], bf16, name=f"tz{h}") for h in range(HPC)]
        vt = [
            persist.tile([128, VTW * N_CHUNKS], bf16, name=f"vt{h}")
            for h in range(HPC)
        ]
        for h in range(HPC):
            nc.gpsimd.memset(vt[h], 1.0)

        sc_pool = ctx.enter_context(tc.tile_pool(name="sc", bufs=3, space="PSUM"))
        ctx_pool = ctx.enter_context(tc.tile_pool(name="ctxp", bufs=2, space="PSUM"))
        ex_pool = ctx.enter_context(tc.tile_pool(name="ex", bufs=5))
        ob_pool = ctx.enter_context(tc.tile_pool(name="obp", bufs=3))

        def cp(i, dst, src):
            if i % 2 == 0:
                nc.vector.tensor_copy(dst, src)
            else:
                nc.scalar.copy(dst, src)

        # ---- projections ----
        ncp = 0
        for h in range(HPC):
            for t in range(4):
                pq = sc_pool.tile([128, 2 * NQB], f32, name="pq", tag="sc")
                for u in range(2):
                    nc.tensor.matmul(
                        out=pq[:, ts(u, NQB)],
                        lhsT=win_sb[:, 128 * h : 128 * (h + 1)],
                        rhs=qin_sb[:, 2 * NQB * t + NQB * u : 2 * NQB * t + NQB * (u + 1)],
                        start=True,
                        stop=True,
                    )
                cp(ncp, tz[h][:, ts(t, 2 * NQB)], pq)
                ncp += 1
        # v flipped: out [nk=128, d=32] per chunk; 32 chunks per psum tile
        for h in range(HPC):
            pv = sc_pool.tile([128, 2 * NQB], f32, name="pv", tag="sc")
            for c in range(N_CHUNKS):
                nc.tensor.matmul(
                    out=pv[:, 32 * c : 32 * (c + 1)],
                    lhsT=kin_sb[:, ts(c, NKC)],
                    rhs=win_sb[:, 256 + 32 * h : 256 + 32 * (h + 1)],
                    start=True,
                    stop=True,
                )
            src3 = pv.rearrange("p (c w) -> p c w", c=N_CHUNKS)
            dst3 = vt[h].rearrange("p (c w) -> p c w", c=N_CHUNKS)
            cp(ncp, dst3[:, :, 0:32], src3)
            ncp += 1

        # ---- attention ----
        exp_sched = _exp_schedule(HPC * N_BLOCKS * (N_CHUNKS // 2))
        nexp = 0
        for h in range(HPC):
            for b in range(N_BLOCKS):
                ctx_ps = ctx_pool.tile([128, 4 * VTW], f32, name="ctx_ps")
                for cc in range(N_CHUNKS // 2):
                    sc = sc_pool.tile([128, 2 * NQB], f32, name="sc", tag="sc")
                    for u in range(2):
                        c = 2 * cc + u
                        nc.tensor.matmul(
                            out=sc[:, ts(u, NQB)],
                            lhsT=kin_sb[:, ts(c, NKC)],
                            rhs=tz[h][:, ts(b, NQB)],
                            start=True,
                            stop=True,
                        )
                    ex = ex_pool.tile([128, 2 * NQB], bf16, name="ex")
                    eng = exp_sched[nexp]
                    nexp += 1
                    if eng == "A":
                        nc.scalar.activation(ex, sc, AF.Exp, scale=ln2)
                    else:
                        nc.vector.tensor_scalar(
                            ex.bitcast(i16), sc, EXP2_A, EXP2_B,
                            op0=ALU.mult, op1=ALU.add,
                        )
                    for u in range(2):
                        c = 2 * cc + u
                        for j in range(4):
                            # NOTE: start=True clears has_written BANK-wide,
                            # so only the tile's very first matmul may set it
                            # (the bit-clear makes every region's first write
                            # an overwrite, later writes accumulate).
                            nc.tensor.matmul(
                                out=ctx_ps[:, ts(j, VTW)],
                                lhsT=ex[:, NQB * u + NKC * j : NQB * u + NKC * (j + 1)],
                                rhs=vt[h][:, ts(c, VTW)],
                                start=(c == 0 and j == 0),
                                stop=(c == N_CHUNKS - 1 and j == 3),
                                skip_group_check=True,
                            )
                ob = ob_pool.tile([128, 4 * VTW], f32, name="ob")
                cp(ncp, ob, ctx_ps)
                ncp += 1
                nc.sync.dma_start(out=out_ctx[h * N_BLOCKS + b], in_=ob)

    nc.compile()
    return nc


def _shard_inputs(query, key, Wq, Wk, Wv):
    query = _f32(query).reshape(B, C, N)
    key = _f32(key).reshape(B, C, N)
    Wq, Wk, Wv = _f32(Wq), _f32(Wk), _f32(Wv)

    scale = math.log2(math.e) / math.sqrt(DH)
    in_maps = []
    for core in range(NCORES):
        b, half = core // 2, core % 2
        win = np.zeros((128, 320), np.float32)
        for hl in range(HPC):
            ch0 = 64 * half + 32 * hl
            wq_h = Wq[ch0 : ch0 + 32, :]  # [32, 128]
            wk_h = Wk[ch0 : ch0 + 32, :]
            win[:, 128 * hl : 128 * (hl + 1)] = scale * (wq_h.T @ wk_h)
            win[:, 256 + 32 * hl : 256 + 32 * (hl + 1)] = Wv[ch0 : ch0 + 32, :].T
        in_maps.append(
            {
                "qin": _bf16(query[b]),
                "kin": _bf16(key[b]),
                "win": _bf16(win),
            }
        )
    return in_maps


def _run(in_maps, trace=False):
    from concourse import bass_utils

    nc = _build_program()
    return bass_utils.run_bass_kernel_spmd(
        nc, in_maps, core_ids=list(range(NCORES)), trace=trace
    )


def _assemble(results):
    out = np.empty((B, C, N), np.float32)
    for core in range(NCORES):
        b, half = core // 2, core % 2
        r = results[core]
        t = np.asarray(r["out_ctx"], np.float32)  # [16, 128, 132]
        t = t.reshape(HPC, N_BLOCKS, 128, 4, VTW)
        ctx = t[..., :32]  # [h, b8, p, j, d]
        den = t[..., 32]  # [h, b8, p, j]
        # nq index = b8*512 + j*128 + p -> order (b8, j, p)
        ctx = np.transpose(ctx, (0, 1, 3, 2, 4)).reshape(HPC, N, 32)
        den = np.transpose(den, (0, 1, 3, 2)).reshape(HPC, N)
        for hl in range(HPC):
            ch0 = 64 * half + 32 * hl
            out[b, ch0 : ch0 + 32, :] = (ctx[hl] / den[hl][:, None]).T
    return out.reshape(B, C, HS, WS)


def kernel(query, key, Wq, Wk, Wv):
    in_maps = _shard_inputs(query, key, Wq, Wk, Wv)
    res = _run(in_maps)
    return _assemble(res.results)
